# revision 12
# baseline (speedup 1.0000x reference)
"""Trainium2 Bass kernel for nn_AM2P_55113020342736 (retrieval_knn).

Math: the reference collapses to a single combined prototype vector v[C]:
  s_fg[b,h,w] = (q[b,:,h,w] . v) / max(||q[b,:,h,w]||, 1e-12)
  logits = stack(-s_fg/T, s_fg/T)
where
  v = BETA/T * Ghat + (1-BETA)/T * sum_m w_m * Phat_m
  Ghat   = G / max(||G||, 1e-12*(fg+EPS)),          G[c] = sum_{s,hw} sf*mask
  Phat_m = Fm / max(||Fm||, 1e-12*(msum_m+EPS)),    Fm[c] = windowed masked sum
(the msum/fg denominators cancel under l2 normalization).

Sharding: channel-parallel for the support statistics (each core owns 48 of
384 channels, full spatial extent -> complete per-channel prototype entries),
plus data-parallel over the query batch (1 query image per core). Tiny
AllReduce (65 floats, prototype norms) + AllGather (48->384 floats, v vector)
in the middle; all heavy tensors are read exactly once from HBM per core.

Support is processed in h-halves ([48ch, 48rows, 96] tiles, all partition-0
based, masked in place via one fused DVE multiply+reduce per half); anchor
windows crossing the h=48 boundary get their two partials summed with one
extra DVE add.

Anchor windows / mask counts / weights are integer bookkeeping derived from
int32 inputs; they are folded into per-anchor reduce slices and two 65-float
coefficient vectors on the host at trace time (the float math all runs on
device).
"""

import numpy as np

S, C, H, W = 4, 384, 96, 96
B, M = 8, 64
HW = H * W
NCORES = 8
CS = C // NCORES          # 48 channels per core
NP = M + 1                # 64 local prototypes + 1 global
HH = H // 2               # 48 rows per h-half
FH = HH * W               # 4608 elements per half
RADII = (4, 8, 16)
BETA, TEMP, EPS = 0.3, 0.07, 1e-6
NJ = 18                   # hw chunks of 512
JW = 512
NK = 3                    # c chunks of 128


def _build_program(windows):
    """windows: list of (s, y1, y2, x1, x2) per anchor (inclusive bounds)."""
    import concourse.bass as bass
    import concourse.bacc as bacc
    import concourse.mybir as mybir
    import concourse.tile as tile

    f32 = mybir.dt.float32
    bf16 = mybir.dt.bfloat16
    add = mybir.AluOpType.add
    mult = mybir.AluOpType.mult
    amax = mybir.AluOpType.max
    bypass = mybir.AluOpType.bypass
    XY = mybir.AxisListType.XY
    X = mybir.AxisListType.X

    nc = bacc.Bacc()
    qf = nc.declare_dram_parameter("qf", [C, HW], f32, isOutput=False)
    sf = nc.declare_dram_parameter("sf", [S, CS, 2, FH], f32, isOutput=False)
    maskf = nc.declare_dram_parameter("maskf", [S, 2, FH], f32, isOutput=False)
    tiny = nc.declare_dram_parameter("tiny", [NP, 1], f32, isOutput=False)
    wcoef = nc.declare_dram_parameter("wcoef", [NP, 1], f32, isOutput=False)
    ident = nc.declare_dram_parameter("ident", [CS, CS], f32, isOutput=False)
    out = nc.declare_dram_parameter("out", [2, NJ, JW], f32, isOutput=True)

    groups = [list(range(NCORES))]

    # split each anchor window into per-half row ranges
    # halves[(s, h)] -> list of (m, ylo, yhi, x1, x2, is_second_part)
    parts_by_sh = {(s, h): [] for s in range(S) for h in range(2)}
    n_parts = [0] * M
    for m, (s, y1, y2, x1, x2) in enumerate(windows):
        for h in range(2):
            ylo = max(y1 - h * HH, 0)
            yhi = min(y2 - h * HH, HH - 1)
            if ylo > yhi:
                continue
            parts_by_sh[(s, h)].append((m, ylo, yhi, x1, x2, n_parts[m]))
            n_parts[m] += 1

    with tile.TileContext(nc) as tc:
        with (
            tc.tile_pool(name="dram", bufs=1, space="DRAM") as dram,
            tc.tile_pool(name="constp", bufs=1) as constp,
            tc.tile_pool(name="supp", bufs=1) as supp,
            tc.tile_pool(name="qres", bufs=1) as qres,
            tc.tile_pool(name="work", bufs=3) as work,
            tc.tile_pool(name="psum", bufs=2, space=bass.MemorySpace.PSUM) as psum,
            tc.tile_pool(name="psum1", bufs=1, space=bass.MemorySpace.PSUM) as psum1,
        ):
            # ---- constants ----
            ident_sb = constp.tile([CS, CS], f32)
            nc.sync.dma_start(out=ident_sb[:], in_=ident[:])
            tiny_sb = constp.tile([NP, 1], f32)
            nc.sync.dma_start(out=tiny_sb[:], in_=tiny[:])
            wcoef_sb = constp.tile([NP, 1], f32)
            nc.sync.dma_start(out=wcoef_sb[:], in_=wcoef[:])
            ones48 = constp.tile([CS, 1], f32)
            nc.vector.memset(ones48[:], 1.0)
            ones128 = constp.tile([128, 1], bf16)
            nc.vector.memset(ones128[:], 1.0)

            # ---- support phase: prototype matrix F [CS, NP] ----
            F = constp.tile([CS, NP], f32)
            Gparts = constp.tile([CS, 2 * S], f32)

            for s in range(S):
                for h in range(2):
                    fs = supp.tile([CS, HH, W], f32, tag="fs", bufs=2)
                    nc.sync.dma_start(out=fs[:], in_=sf[s][:, h])
                    mrow = supp.tile([1, FH], f32, tag="mrow", bufs=1)
                    nc.sync.dma_start(out=mrow[:], in_=maskf[s][h].unsqueeze(0))
                    mrep = supp.tile([CS, FH], f32, tag="mrep", bufs=2)
                    nc.gpsimd.partition_broadcast(mrep[:], mrow[:])
                    # fs *= mask (in place); then Gpart = sum over this half
                    # (ScalarE copy-with-accumulate; the copy target is the
                    # dead mrep tile)
                    fsflat = fs[:].rearrange("a b c -> a (b c)")
                    nc.vector.tensor_tensor(
                        out=fsflat, in0=fsflat, in1=mrep[:], op=mult
                    )
                    nc.scalar.activation(
                        mrep[:], fsflat, mybir.ActivationFunctionType.Copy,
                        accum_out=Gparts[:, 2 * s + h : 2 * s + h + 1],
                    )
                    for m, ylo, yhi, x1, x2, part in parts_by_sh[(s, h)]:
                        if part == 0:
                            nc.vector.tensor_reduce(
                                out=F[:, m : m + 1],
                                in_=fs[:, ylo : yhi + 1, x1 : x2 + 1],
                                axis=XY, op=add,
                            )
                        else:
                            t2 = supp.tile([CS, 1], f32, tag="t2", bufs=2)
                            nc.vector.tensor_reduce(
                                out=t2[:],
                                in_=fs[:, ylo : yhi + 1, x1 : x2 + 1],
                                axis=XY, op=add,
                            )
                            nc.vector.tensor_tensor(
                                out=F[:, m : m + 1], in0=F[:, m : m + 1],
                                in1=t2[:], op=add,
                            )
            # global-proto column = sum of per-(sample,half) masked sums
            nc.vector.tensor_reduce(out=F[:, M : M + 1], in_=Gparts[:], axis=X, op=add)

            # ---- prototype norms -> AllReduce ----
            F2 = constp.tile([CS, NP], f32)
            nc.scalar.square(F2[:], F[:])
            n2ps = psum1.tile([1, NP], f32, tag="n2ps")
            nc.tensor.matmul(n2ps[:], ones48[:], F2[:], start=True, stop=True)
            n2row = constp.tile([1, NP], f32)
            nc.scalar.copy(n2row[:], n2ps[:])
            ar_in = dram.tile([NP, 1], f32)
            ar_out = dram.tile([NP, 1], f32, addr_space="Shared")
            nc.sync.dma_start(out=ar_in[:], in_=n2row[:])
            nc.gpsimd.collective_compute(
                "AllReduce", add, replica_groups=groups,
                ins=[ar_in.opt()], outs=[ar_out.opt()],
            )

            # ---- coef_m = wcoef_m / max(sqrt(n2_m), tiny_m) ----
            n2 = constp.tile([NP, 1], f32)
            nc.sync.dma_start(out=n2[:], in_=ar_out[:])
            nrm = constp.tile([NP, 1], f32)
            nc.scalar.sqrt(nrm[:], n2[:])
            nc.vector.tensor_tensor(out=nrm[:], in0=nrm[:], in1=tiny_sb[:], op=amax)
            rcp = constp.tile([NP, 1], f32)
            nc.vector.reciprocal(rcp[:], nrm[:])
            coef = constp.tile([NP, 1], f32)
            nc.vector.tensor_tensor(out=coef[:], in0=rcp[:], in1=wcoef_sb[:], op=mult)

            # ---- v slice = coef^T @ F^T  -> AllGather ----
            ftps = psum1.tile([NP, CS], f32, tag="ftps")
            nc.tensor.transpose(ftps[:], F[:], ident_sb[:])
            FT = constp.tile([NP, CS], f32)
            nc.scalar.copy(FT[:], ftps[:])
            vps = psum1.tile([1, CS], f32, tag="vps")
            nc.tensor.matmul(vps[:], coef[:], FT[:], start=True, stop=True)
            vrow = constp.tile([1, CS], f32)
            nc.scalar.copy(vrow[:], vps[:])
            ag_in = dram.tile([CS], f32)
            ag_out = dram.tile([NK, 128], f32, addr_space="Shared")
            nc.sync.dma_start(out=ag_in[:], in_=vrow[:])
            nc.gpsimd.collective_compute(
                "AllGather", bypass, replica_groups=groups,
                ins=[ag_in.opt()], outs=[ag_out.opt()],
            )
            vfull = constp.tile([128, NK], f32)
            for k in range(NK):
                nc.sync.dma_start(out=vfull[:, k : k + 1], in_=ag_out[k].unsqueeze(1))
            vb = constp.tile([128, NK], bf16)
            nc.vector.tensor_copy(vb[:], vfull[:])

            # ---- query phase ----
            qb = [qres.tile([128, HW], bf16, name=f"qb{k}") for k in range(NK)]
            norm2 = constp.tile([NJ, JW], f32)
            dots = constp.tile([NJ, JW], f32)
            for j in range(NJ):
                n2p = psum.tile([1, JW], f32, tag="n2p")
                dtp = psum.tile([1, JW], f32, tag="dtp")
                for k in range(NK):
                    qt = work.tile([128, JW], f32, tag="qt")
                    nc.sync.dma_start(
                        out=qt[:],
                        in_=qf[k * 128 : (k + 1) * 128, j * JW : (j + 1) * JW],
                    )
                    nc.vector.tensor_copy(qb[k][:, j * JW : (j + 1) * JW], qt[:])
                    q2 = work.tile([128, JW], bf16, tag="q2")
                    nc.scalar.square(q2[:], qt[:])
                    nc.tensor.matmul(
                        n2p[:], ones128[:], q2[:], start=(k == 0), stop=(k == NK - 1)
                    )
                    nc.tensor.matmul(
                        dtp[:], vb[:, k : k + 1], qb[k][:, j * JW : (j + 1) * JW],
                        start=(k == 0), stop=(k == NK - 1),
                    )
                tmpn = work.tile([1, JW], f32, tag="tmpn")
                nc.scalar.copy(tmpn[:], n2p[:])
                nc.sync.dma_start(out=norm2[j : j + 1, :], in_=tmpn[:])
                tmpd = work.tile([1, JW], f32, tag="tmpd")
                nc.vector.tensor_copy(tmpd[:], dtp[:])
                nc.sync.dma_start(out=dots[j : j + 1, :], in_=tmpd[:])

            # ---- epilogue: s1 = dots / max(sqrt(norm2), 1e-12); s0 = -s1 ----
            den = constp.tile([NJ, JW], f32)
            nc.scalar.sqrt(den[:], norm2[:])
            nc.vector.tensor_scalar_max(den[:], den[:], 1e-12)
            rden = constp.tile([NJ, JW], f32)
            nc.vector.reciprocal(rden[:], den[:])
            s1 = constp.tile([NJ, JW], f32)
            nc.vector.tensor_tensor(out=s1[:], in0=dots[:], in1=rden[:], op=mult)
            s0 = constp.tile([NJ, JW], f32)
            nc.scalar.mul(s0[:], s1[:], -1.0)
            nc.sync.dma_start(out=out[1], in_=s1[:])
            nc.sync.dma_start(out=out[0], in_=s0[:])

    nc.finalize()
    return nc


def prepare(support_feats, support_masks, query_feats, anchor_pos,
            anchor_sample, anchor_radius):
    """Host prep: returns (nc, in_maps)."""
    mask = support_masks[:, 0].astype(np.float32)          # [S,H,W]
    fg = float(np.float32(mask.sum()))

    # integral image of mask for windowed fg counts (host, int bookkeeping)
    ii = np.zeros((S, H + 1, W + 1), np.float64)
    ii[:, 1:, 1:] = mask.astype(np.float64).cumsum(1).cumsum(2)

    windows, msums = [], []
    for m in range(M):
        y, x = int(anchor_pos[m, 0]), int(anchor_pos[m, 1])
        s = int(anchor_sample[m])
        r = RADII[int(anchor_radius[m])]
        y1, y2 = max(y - r, 0), min(y + r, H - 1)
        x1, x2 = max(x - r, 0), min(x + r, W - 1)
        windows.append((s, y1, y2, x1, x2))
        msums.append(ii[s, y2 + 1, x2 + 1] - ii[s, y1, x2 + 1]
                     - ii[s, y2 + 1, x1] + ii[s, y1, x1])
    msums = np.asarray(msums, np.float32)

    # reference's double weight normalization, in f32 like the reference
    lw = msums / (np.float32(msums.sum()) + np.float32(EPS))
    w = lw / (np.float32(lw.sum()) + np.float32(EPS))

    tiny = np.empty((NP, 1), np.float32)
    tiny[:M, 0] = 1e-12 * (msums + np.float32(EPS))
    tiny[M, 0] = 1e-12 * (fg + EPS)
    wcoef = np.empty((NP, 1), np.float32)
    wcoef[:M, 0] = (1.0 - BETA) * w / TEMP
    wcoef[M, 0] = BETA / TEMP

    nc = _build_program(windows)

    maskf = np.ascontiguousarray(mask.reshape(S, 2, FH))
    ident = np.eye(CS, dtype=np.float32)
    qfv = query_feats.reshape(B, C, HW)
    in_maps = []
    for i in range(NCORES):
        sfc = np.ascontiguousarray(support_feats[:, i * CS : (i + 1) * CS])
        in_maps.append({
            "qf": np.ascontiguousarray(qfv[i]),
            "sf": sfc.reshape(S, CS, 2, FH),
            "maskf": maskf,
            "tiny": tiny,
            "wcoef": wcoef,
            "ident": ident,
        })

    return nc, in_maps


def assemble(results):
    outs = [np.asarray(results[i]["out"], np.float32).reshape(2, H, W)
            for i in range(NCORES)]
    return np.stack(outs, axis=0)


def kernel(support_feats, support_masks, query_feats, anchor_pos,
           anchor_sample, anchor_radius):
    from concourse.bass_utils import run_bass_kernel_spmd

    nc, in_maps = prepare(support_feats, support_masks, query_feats,
                          anchor_pos, anchor_sample, anchor_radius)
    res = run_bass_kernel_spmd(nc, in_maps, core_ids=list(range(NCORES)))
    return assemble(res.results)


if __name__ == "__main__":
    pass


# revision 14
# speedup vs baseline: 1.9597x; 1.9597x over previous
"""Trainium2 Bass kernel for nn_AM2P_55113020342736 (retrieval_knn).

Math: the reference collapses to a single combined prototype vector v[C]:
  s_fg[b,h,w] = (q[b,:,h,w] . v) / max(||q[b,:,h,w]||, 1e-12)
  logits = stack(-s_fg/T, s_fg/T)
where
  v = BETA/T * Ghat + (1-BETA)/T * sum_m w_m * Phat_m
  Ghat   = G / max(||G||, 1e-12*(fg+EPS)),          G[c] = sum_{s,hw} sf*mask
  Phat_m = Fm / max(||Fm||, 1e-12*(msum_m+EPS)),    Fm[c] = windowed masked sum
(the msum/fg denominators cancel under l2 normalization).

Sharding:
- Support statistics: each core owns one (sample, h-half) slice with ALL 384
  channels, host-transposed to [hw=4608, C]. The prototype matrix
  F[65, 384] = W^T @ feats is computed as 36 TensorE matmuls accumulating in
  one PSUM bank, where W[hw, m] = mask AND window indicator (host-built 0/1
  f32 from the int32 mask/anchor inputs). One 100KB AllReduce then gives
  every core the complete F; each core redundantly derives the tiny v.
- Query path: data-parallel, 1 query image per core, kept f32-resident in
  SBUF; per-pixel dot & squared-norm contract over C via M=1 matmuls.

All float math runs on device; the host only slices/transposes inputs and
builds integer-derived 0/1 weight matrices and two 65-float coefficient
vectors.
"""

import numpy as np

S, C, H, W = 4, 384, 96, 96
B, M = 8, 64
HW = H * W
NCORES = 8
NP = M + 1                # 64 local prototypes + 1 global
HH = H // 2               # 48 rows per h-half
FH = HH * W               # 4608 support rows per core slice
RADII = (4, 8, 16)
BETA, TEMP, EPS = 0.3, 0.07, 1e-6
NK = 3                    # query c chunks of 128
JW = 512                  # matmul free width
QP = 1536                 # query DMA piece (3 x JW)
NPIECE = HW // QP         # 6
NSUP = FH // (3 * 128)    # 12 support DMA chunks of [128, 3, C]


def _build_program():
    import concourse.bass as bass
    import concourse.bacc as bacc
    import concourse.mybir as mybir
    import concourse.tile as tile

    f32 = mybir.dt.float32
    bf16 = mybir.dt.bfloat16
    add = mybir.AluOpType.add
    mult = mybir.AluOpType.mult
    amax = mybir.AluOpType.max

    nc = bacc.Bacc()
    qf = nc.declare_dram_parameter("qf", [C, HW], f32, isOutput=False)
    sft = nc.declare_dram_parameter("sft", [NSUP, 128, 3, C], f32, isOutput=False)
    wmat = nc.declare_dram_parameter("wmat", [NSUP, 128, 3, NP], f32, isOutput=False)
    tiny = nc.declare_dram_parameter("tiny", [NP, 1], f32, isOutput=False)
    wcoef = nc.declare_dram_parameter("wcoef", [NP, 1], f32, isOutput=False)
    out = nc.declare_dram_parameter("out", [2, HW // JW, JW], f32, isOutput=True)

    groups = [list(range(NCORES))]
    NJ = HW // JW  # 18

    with tile.TileContext(nc) as tc:
        with (
            tc.tile_pool(name="dram", bufs=1, space="DRAM") as dram,
            tc.tile_pool(name="constp", bufs=1) as constp,
            tc.tile_pool(name="qres", bufs=1) as qres,
            tc.tile_pool(name="work", bufs=3) as work,
            tc.tile_pool(name="psum", bufs=2, space=bass.MemorySpace.PSUM) as psum,
            tc.tile_pool(name="psum1", bufs=1, space=bass.MemorySpace.PSUM) as psum1,
        ):
            # ---- constants ----
            tiny_sb = constp.tile([NP, 1], f32)
            nc.sync.dma_start(out=tiny_sb[:], in_=tiny[:])
            wcoef_sb = constp.tile([NP, 1], f32)
            nc.sync.dma_start(out=wcoef_sb[:], in_=wcoef[:])
            ones128 = constp.tile([128, 1], bf16)
            nc.vector.memset(ones128[:], 1.0)

            # ---- support phase: F_partial[NP, C] = W^T @ feats ----
            fps = psum1.tile([NP, C], f32, tag="fps")
            for d in range(NSUP):
                ft = work.tile([128, 3, C], f32, tag="ft")
                nc.sync.dma_start(out=ft[:], in_=sft[d])
                wt = work.tile([128, 3, NP], f32, tag="wt")
                nc.sync.dma_start(out=wt[:], in_=wmat[d])
                for j in range(3):
                    nc.tensor.matmul(
                        fps[:], wt[:, j, :], ft[:, j, :],
                        start=(d == 0 and j == 0),
                        stop=(d == NSUP - 1 and j == 2),
                    )
            fpart = constp.tile([NP, C], f32)
            nc.scalar.copy(fpart[:], fps[:])
            ar_in = dram.tile([NP, C], f32)
            ar_out = dram.tile([NP, C], f32, addr_space="Shared")
            nc.sync.dma_start(out=ar_in[:], in_=fpart[:])
            nc.gpsimd.collective_compute(
                "AllReduce", add, replica_groups=groups,
                ins=[ar_in.opt()], outs=[ar_out.opt()],
            )
            F = constp.tile([NP, C], f32)
            nc.sync.dma_start(out=F[:], in_=ar_out[:])

            # ---- coef_m = wcoef_m / max(||F_m||, tiny_m);  v = coef^T @ F ----
            F2 = constp.tile([NP, C], f32)
            n2 = constp.tile([NP, 1], f32)
            nc.scalar.activation(F2[:], F[:], mybir.ActivationFunctionType.Square,
                                 accum_out=n2[:])
            nrm = constp.tile([NP, 1], f32)
            nc.scalar.sqrt(nrm[:], n2[:])
            nc.vector.tensor_tensor(out=nrm[:], in0=nrm[:], in1=tiny_sb[:], op=amax)
            rcp = constp.tile([NP, 1], f32)
            nc.vector.reciprocal(rcp[:], nrm[:])
            coef = constp.tile([NP, 1], f32)
            nc.vector.tensor_tensor(out=coef[:], in0=rcp[:], in1=wcoef_sb[:], op=mult)
            vps = psum1.tile([1, C], f32, tag="vps")
            nc.tensor.matmul(vps[:], coef[:], F[:], start=True, stop=True)
            vrow = constp.tile([1, C], f32)
            nc.scalar.copy(vrow[:], vps[:])
            # reshape v -> [128, NK] column-per-c-chunk via a DRAM bounce
            vd = dram.tile([NK, 128], f32)
            nc.sync.dma_start(out=vd[:], in_=vrow[:])
            vcol = constp.tile([128, NK], f32)
            for k in range(NK):
                nc.sync.dma_start(out=vcol[:, k : k + 1], in_=vd[k].unsqueeze(1))

            # ---- query phase ----
            qb = [qres.tile([128, HW], f32, name=f"qb{k}") for k in range(NK)]
            norm2 = constp.tile([NJ, JW], f32)
            dots = constp.tile([NJ, JW], f32)
            # stream q pieces; squares + norm2 matmuls (v-independent)
            for p in range(NPIECE):
                q2t = []
                for k in range(NK):
                    qslice = qb[k][:, p * QP : (p + 1) * QP]
                    nc.sync.dma_start(
                        out=qslice, in_=qf[k * 128 : (k + 1) * 128, p * QP : (p + 1) * QP]
                    )
                    q2 = work.tile([128, QP], bf16, tag="q2", bufs=6)
                    nc.vector.tensor_tensor(out=q2[:], in0=qslice, in1=qslice, op=mult)
                    q2t.append(q2)
                for jj in range(QP // JW):
                    j = p * (QP // JW) + jj
                    n2p = psum.tile([1, JW], f32, tag="n2p")
                    for k in range(NK):
                        nc.tensor.matmul(
                            n2p[:], ones128[:], q2t[k][:, jj * JW : (jj + 1) * JW],
                            start=(k == 0), stop=(k == NK - 1),
                        )
                    tmpn = work.tile([1, JW], f32, tag="tmpn")
                    nc.scalar.copy(tmpn[:], n2p[:])
                    nc.sync.dma_start(out=norm2[j : j + 1, :], in_=tmpn[:])
            # dots (gated on v via vcol dependency)
            for j in range(NJ):
                dtp = psum.tile([1, JW], f32, tag="dtp")
                for k in range(NK):
                    nc.tensor.matmul(
                        dtp[:], vcol[:, k : k + 1],
                        qb[k][:, j * JW : (j + 1) * JW],
                        start=(k == 0), stop=(k == NK - 1),
                    )
                tmpd = work.tile([1, JW], f32, tag="tmpd")
                nc.scalar.copy(tmpd[:], dtp[:])
                nc.sync.dma_start(out=dots[j : j + 1, :], in_=tmpd[:])

            # ---- epilogue: s1 = dots / max(sqrt(norm2), 1e-12); s0 = -s1 ----
            den = constp.tile([NJ, JW], f32)
            nc.scalar.sqrt(den[:], norm2[:])
            nc.vector.tensor_scalar_max(den[:], den[:], 1e-12)
            rden = constp.tile([NJ, JW], f32)
            nc.vector.reciprocal(rden[:], den[:])
            s1 = constp.tile([NJ, JW], f32)
            nc.vector.tensor_tensor(out=s1[:], in0=dots[:], in1=rden[:], op=mult)
            s0 = constp.tile([NJ, JW], f32)
            nc.scalar.mul(s0[:], s1[:], -1.0)
            nc.sync.dma_start(out=out[1], in_=s1[:])
            nc.sync.dma_start(out=out[0], in_=s0[:])

    nc.finalize()
    return nc


def prepare(support_feats, support_masks, query_feats, anchor_pos,
            anchor_sample, anchor_radius):
    """Host prep: returns (nc, in_maps)."""
    mask = support_masks[:, 0].astype(np.float32)          # [S,H,W]
    fg = float(np.float32(mask.sum()))

    # integral image of mask for windowed fg counts (host, int bookkeeping)
    ii = np.zeros((S, H + 1, W + 1), np.float64)
    ii[:, 1:, 1:] = mask.astype(np.float64).cumsum(1).cumsum(2)

    windows, msums = [], []
    for m in range(M):
        y, x = int(anchor_pos[m, 0]), int(anchor_pos[m, 1])
        s = int(anchor_sample[m])
        r = RADII[int(anchor_radius[m])]
        y1, y2 = max(y - r, 0), min(y + r, H - 1)
        x1, x2 = max(x - r, 0), min(x + r, W - 1)
        windows.append((s, y1, y2, x1, x2))
        msums.append(ii[s, y2 + 1, x2 + 1] - ii[s, y1, x2 + 1]
                     - ii[s, y2 + 1, x1] + ii[s, y1, x1])
    msums = np.asarray(msums, np.float32)

    # reference's double weight normalization, in f32 like the reference
    lw = msums / (np.float32(msums.sum()) + np.float32(EPS))
    w = lw / (np.float32(lw.sum()) + np.float32(EPS))

    tiny = np.empty((NP, 1), np.float32)
    tiny[:M, 0] = 1e-12 * (msums + np.float32(EPS))
    tiny[M, 0] = 1e-12 * (fg + EPS)
    wcoef = np.empty((NP, 1), np.float32)
    wcoef[:M, 0] = (1.0 - BETA) * w / TEMP
    wcoef[M, 0] = BETA / TEMP

    nc = _build_program()

    qfv = query_feats.reshape(B, C, HW)
    in_maps = []
    for i in range(NCORES):
        s, h = i // 2, i % 2
        # feats slice [C, HH, W] -> transposed [FH, C]
        fsl = support_feats[s, :, h * HH : (h + 1) * HH, :].reshape(C, FH)
        sft = np.ascontiguousarray(fsl.T).reshape(NSUP, 128, 3, C)
        # W[hw, m] = mask AND (hw in window of anchor m with s_m == s);
        # col 64 = mask (global proto)
        msl = mask[s, h * HH : (h + 1) * HH, :]               # [HH, W]
        wm = np.zeros((HH, W, NP), np.float32)
        wm[:, :, M] = msl
        for m, (sm, y1, y2, x1, x2) in enumerate(windows):
            if sm != s:
                continue
            yl = max(y1 - h * HH, 0)
            yh = min(y2 - h * HH, HH - 1)
            if yl > yh:
                continue
            wm[yl : yh + 1, x1 : x2 + 1, m] = msl[yl : yh + 1, x1 : x2 + 1]
        wm = wm.reshape(NSUP, 128, 3, NP)
        in_maps.append({
            "qf": np.ascontiguousarray(qfv[i]),
            "sft": sft,
            "wmat": np.ascontiguousarray(wm),
            "tiny": tiny,
            "wcoef": wcoef,
        })
    return nc, in_maps


def assemble(results):
    outs = [np.asarray(results[i]["out"], np.float32).reshape(2, H, W)
            for i in range(NCORES)]
    return np.stack(outs, axis=0)


def kernel(support_feats, support_masks, query_feats, anchor_pos,
           anchor_sample, anchor_radius):
    from concourse.bass_utils import run_bass_kernel_spmd

    nc, in_maps = prepare(support_feats, support_masks, query_feats,
                          anchor_pos, anchor_sample, anchor_radius)
    res = run_bass_kernel_spmd(nc, in_maps, core_ids=list(range(NCORES)))
    return assemble(res.results)


if __name__ == "__main__":
    pass
